# revision 8
# baseline (speedup 1.0000x reference)
"""GAT 2-layer network (PyG GATConv x2 + log_softmax) on 8 Trainium2 NeuronCores.

Strategy (dst-sharded message passing, SWDGE dma_gather edition):
  - Nodes are 1D-sharded across the 8 cores (6250 nodes/core). Edges (with
    self-loops) are assigned to the core owning their *destination* node.
  - Phase 0 (per core): h = x @ W1.T for the core's node slice (rows of
    exactly 128 bf16 = 256B) written to a DRAM table; per-node adc = h.a_dst
    written to a local [SLICEP, 128] table (cols 0:4).
  - AllGather (via AllToAll) the h table so every core has all nodes' rows.
  - Edge phase (per core): edges sorted by dst, grouped into 128-dst-node
    "dst-blocks", each padded to B 128-edge columns.  Per chunk of dst-blocks:
      * ONE custom dma_gather fetches 512B row-PAIRS by src//2 (int16-safe
        indices), a DVE select by src parity extracts the right 256B row.
      * ONE dma_gather fetches per-edge adc rows from the local table (local
        dst index, int16-safe).  Gathers rotate over SWDGE queues 1..3 so
        descriptor generation runs on three Q7 core-pairs in parallel.
      * per-edge asc = sum(h * a_src) via DVE mult+grouped-reduce.
      * p = exp(leaky_relu(asc_src + adc_dst)); padding edges point at a
        -200 adc sentinel row so p ~ 0.
      * one-hot selection matrix S[e, d] = (dstslot[e] == d) built on DVE;
        PSUM-accumulated matmuls S.T @ [h*p | p] give per-dst sums.
      * drain: out = (sum p h)/denom + b, relu -> layer-2 rows
        [h2 | asc2] and local adc2, written to layer-2 tables.
  - AllToAll the layer-2 table, run the same edge phase with 32 features and
    1 head, finish with log_softmax per node, write the output slice.

The kernel() entry point takes the FULL inputs and returns the FULL output.
"""

import math
from contextlib import ExitStack
from dataclasses import dataclass

import numpy as np

import concourse.bass as bass
import concourse.mybir as mybir
import concourse.tile as tile
from concourse import bacc
from concourse.bass_utils import run_bass_kernel_spmd

F32 = mybir.dt.float32
BF16 = mybir.dt.bfloat16
I16 = mybir.dt.int16
AX = mybir.AxisListType
OP = mybir.AluOpType
AF = mybir.ActivationFunctionType

SENT_ADC = -200.0  # sentinel adc: exp(lrelu(asc-200)) ~ 0


@dataclass(frozen=True)
class Cfg:
    N: int = 50000
    F_IN: int = 256
    HID: int = 32
    HEADS: int = 4
    CLASSES: int = 32
    NC: int = 8
    TILE_N: int = 512   # phase-0 node tile
    CHUNK_DB: int = 2   # dst-blocks per gather chunk
    B: int = 18         # uniform 128-edge columns per dst-block (data dep.)

    @property
    def F(self):  # layer-1 feature width == 128
        return self.HID * self.HEADS

    @property
    def NPC(self):
        return self.N // self.NC

    @property
    def T0(self):
        return math.ceil(self.NPC / self.TILE_N)

    @property
    def SLICE(self):
        return self.T0 * self.TILE_N

    @property
    def NB(self):  # dst-blocks per core
        return math.ceil(self.NPC / 128)

    @property
    def SLICEP(self):  # per-core table slice incl sentinel row
        return self.SLICE + 1

    @property
    def NTAB(self):  # node-table rows (must be even for pair-gather)
        n = self.NC * self.SLICEP
        return n + (n % 2)

    @property
    def NCH(self):
        return math.ceil(self.NB / self.CHUNK_DB)

    @property
    def NBB(self):
        return self.NB * self.B

    @property
    def FR(self):  # layer-1 rhs width: h*p(F) + p(HEADS)
        return self.F + self.HEADS

    @property
    def FR2(self):  # layer-2 rhs width
        return self.CLASSES + 1


def compute_B(dst, cfg: Cfg):
    """Max 128-edge columns needed by any (core, dst-block)."""
    NPC, NB = cfg.NPC, cfg.NB
    core = dst // NPC
    blk = (dst % NPC) // 128
    cnt = np.bincount(core * NB + blk, minlength=cfg.NC * NB)
    return int(np.max((cnt + 127) // 128))


def build_layout(src, dst, cfg: Cfg):
    """Per-core edge arrays.

    Edge stream order within a chunk: t = col*128 + p (the dma_gather output
    row order).  Returns per-core:
      src16 [128, NBB*8] i16 : (src_coord >> 1), wrapped 16-partition layout
      adc16 [128, NBB*8] i16 : local dst row (or sentinel), wrapped
      dsl   [128, NBB]  f32 : dst slot within dst-block (0..127)
      par/parn [128, NBB] f32 : src parity and 1-parity
    """
    NC, NPC, NB, B = cfg.NC, cfg.NPC, cfg.NB, cfg.B
    SLICEP = cfg.SLICEP
    NBB = NB * B

    core = dst // NPC
    local = dst % NPC
    blk = local // 128
    seg = core * NB + blk
    order = np.argsort(seg, kind="stable")
    seg_s = seg[order]
    src_s = src[order]
    local_s = local[order]

    cnt = np.bincount(seg, minlength=NC * NB)
    seg_start = np.zeros(NC * NB + 1, dtype=np.int64)
    np.cumsum(cnt, out=seg_start[1:])
    pos = np.arange(len(dst)) - seg_start[seg_s]
    assert pos.max() < B * 128, "B too small for this edge distribution"

    c_s = seg_s // NB
    blk_s = seg_s % NB
    p_s = (pos % 128).astype(np.int64)
    col_s = (blk_s * B + pos // 128).astype(np.int64)

    src_core = src_s // NPC
    src_local = src_s % NPC
    src_coord = (src_core * SLICEP + src_local).astype(np.int64)

    # dense per-core arrays; pads: src_coord=0, adc->sentinel(SLICE), dsl=0
    srcc = np.zeros((NC, 128, NBB), dtype=np.int64)
    adci = np.full((NC, 128, NBB), cfg.SLICE, dtype=np.int64)
    dsl = np.zeros((NC, 128, NBB), dtype=np.float32)
    srcc[c_s, p_s, col_s] = src_coord
    adci[c_s, p_s, col_s] = local_s
    dsl[c_s, p_s, col_s] = (local_s - blk_s * 128).astype(np.float32)

    par = (srcc & 1).astype(np.float32)
    src16 = (srcc >> 1).astype(np.int16)
    adc16 = adci.astype(np.int16)

    # wrap into the dma_gather idx layout: stream t = col*128 + p;
    # wrapped[t % 16, t // 16], replicated to all 128 partitions.  Chunks
    # start at column (=128-edge) multiples so per-chunk slices stay aligned.
    p_idx = np.arange(128)
    c_idx = np.arange(NBB)
    t = (c_idx[None, :] * 128 + p_idx[:, None])        # [128, NBB]
    wp = (t % 16).ravel()
    wf = (t // 16).ravel()

    def wrap_fast(a16):
        out = np.zeros((NC, 16, NBB * 8), dtype=np.int16)
        flat = a16.reshape(NC, 128 * NBB)
        out[:, wp, wf] = flat
        return np.tile(out, (1, 8, 1))

    return wrap_fast(src16), wrap_fast(adc16), dsl, par, 1.0 - par


def pack_consts(W1, a_src1, a_dst1, b1, W2, a_src2, a_dst2, b2, cfg: Cfg):
    F, H, HID, C, F_IN = cfg.F, cfg.HEADS, cfg.HID, cfg.CLASSES, cfg.F_IN
    KT = F_IN // 128
    consts = {}
    w1t = W1.T.reshape(KT, 128, F).transpose(1, 0, 2)
    consts["w1t"] = w1t.astype(np.float32)
    # a_dst blockdiag [F, H]
    adst = np.zeros((F, H), dtype=np.float32)
    for h in range(H):
        adst[h * HID:(h + 1) * HID, h] = a_dst1[h]
    consts["adst"] = adst
    # a_src replicated [128, F]
    consts["arep"] = np.tile(a_src1.reshape(1, F), (128, 1)).astype(np.float32)
    # W2ext [F, C+2]: W2.T | (a_src2@W2).T | (a_dst2@W2).T
    w2e = np.concatenate(
        [W2.T.astype(np.float64),
         (a_src2 @ W2).T.astype(np.float64),
         (a_dst2 @ W2).T.astype(np.float64)], axis=1)
    consts["w2e"] = w2e.astype(np.float32)
    consts["rconst"] = np.tile(np.arange(128, dtype=np.float32)[None, :], (128, 1))
    consts["ident"] = np.eye(128, dtype=np.float32)
    consts["b1rep"] = np.tile(b1[None, :].astype(np.float32), (128, 1))
    consts["b2rep"] = np.tile(b2[None, :].astype(np.float32), (128, 1))
    consts["sent"] = np.full((1, 128), SENT_ADC, dtype=np.float32)
    consts["zrow"] = np.zeros((1, 128), dtype=np.float32)
    return consts


def build_in_maps(x, edge_index, W1, a_src1, a_dst1, b1, W2, a_src2, a_dst2, b2,
                  cfg: Cfg):
    N, NC, NPC, SLICE = cfg.N, cfg.NC, cfg.NPC, cfg.SLICE
    loops = np.arange(N, dtype=edge_index.dtype)
    src = np.concatenate([np.asarray(edge_index[0]), loops]).astype(np.int64)
    dst = np.concatenate([np.asarray(edge_index[1]), loops]).astype(np.int64)

    src16, adc16, dsl, par, parn = build_layout(src, dst, cfg)
    consts = pack_consts(np.asarray(W1), np.asarray(a_src1), np.asarray(a_dst1),
                         np.asarray(b1), np.asarray(W2), np.asarray(a_src2),
                         np.asarray(a_dst2), np.asarray(b2), cfg)

    xT = np.zeros((cfg.F_IN, NC * SLICE), dtype=np.float32)
    xv = np.asarray(x).T
    for c in range(NC):
        xT[:, c * SLICE:c * SLICE + NPC] = xv[:, c * NPC:(c + 1) * NPC]

    import ml_dtypes
    to_bf16 = lambda a: a.astype(ml_dtypes.bfloat16)

    in_maps = []
    for c in range(NC):
        m = {
            "xt": to_bf16(xT[:, c * SLICE:(c + 1) * SLICE]),
            "src16": src16[c],
            "adc16": adc16[c],
            "dsl": to_bf16(dsl[c]),
            "par": to_bf16(par[c]),
            "parn": to_bf16(parn[c]),
            "w1t": to_bf16(consts["w1t"]),
            "adst": to_bf16(consts["adst"]),
            "arep": to_bf16(consts["arep"]),
            "w2e": to_bf16(consts["w2e"]),
            "rconst": to_bf16(consts["rconst"]),
            "ident": to_bf16(consts["ident"]),
            "b1rep": consts["b1rep"],
            "b2rep": consts["b2rep"],
            "sent": to_bf16(consts["sent"]),
            "zrow": to_bf16(consts["zrow"]),
        }
        in_maps.append(m)
    return in_maps


def build_nc(cfg: Cfg):
    """Build + compile the SPMD program (identical for all cores)."""
    N, NC, NPC, NB, B, SLICE, T0 = (cfg.N, cfg.NC, cfg.NPC, cfg.NB, cfg.B,
                                    cfg.SLICE, cfg.T0)
    F, H, C, F_IN = cfg.F, cfg.HEADS, cfg.CLASSES, cfg.F_IN
    FR, FR2 = cfg.FR, cfg.FR2
    TILE_N, CHUNK_DB, NCH = cfg.TILE_N, cfg.CHUNK_DB, cfg.NCH
    NBB = NB * B
    KT = F_IN // 128
    SLICEP = cfg.SLICEP
    NTAB = cfg.NTAB
    LASTV = NPC - (NB - 1) * 128
    CTMAX = CHUNK_DB * B

    nc = bacc.Bacc("TRN2", target_bir_lowering=False, debug=False,
                   num_devices=NC, num_swdge_queues=4)

    dt = nc.dram_tensor
    xt_d = dt("xt", [F_IN, SLICE], BF16, kind="ExternalInput").ap()
    src16_d = dt("src16", [128, NBB * 8], I16, kind="ExternalInput").ap()
    adc16_d = dt("adc16", [128, NBB * 8], I16, kind="ExternalInput").ap()
    dsl_d = dt("dsl", [128, NBB], BF16, kind="ExternalInput").ap()
    par_d = dt("par", [128, NBB], BF16, kind="ExternalInput").ap()
    parn_d = dt("parn", [128, NBB], BF16, kind="ExternalInput").ap()
    w1t_d = dt("w1t", [128, KT, F], BF16, kind="ExternalInput").ap()
    adst_d = dt("adst", [F, H], BF16, kind="ExternalInput").ap()
    arep_d = dt("arep", [128, F], BF16, kind="ExternalInput").ap()
    w2e_d = dt("w2e", [F, C + 2], BF16, kind="ExternalInput").ap()
    rconst_d = dt("rconst", [128, 128], BF16, kind="ExternalInput").ap()
    ident_d = dt("ident", [128, 128], BF16, kind="ExternalInput").ap()
    b1rep_d = dt("b1rep", [128, F], F32, kind="ExternalInput").ap()
    b2rep_d = dt("b2rep", [128, C], F32, kind="ExternalInput").ap()
    sent_d = dt("sent", [1, 128], BF16, kind="ExternalInput").ap()
    zrow_d = dt("zrow", [1, 128], BF16, kind="ExternalInput").ap()
    out_d = dt("out", [NPC, C], F32, kind="ExternalOutput").ap()

    rg = [list(range(NC))]

    with tile.TileContext(nc) as tc, ExitStack() as top:
        dram = top.enter_context(tc.tile_pool(name="dram", bufs=1, space="DRAM"))
        hext_loc = dram.tile([SLICEP, 128], BF16)
        h2_loc = dram.tile([SLICEP, 128], BF16)
        adc1_t = dram.tile([SLICEP, 128], BF16)
        adc2_t = dram.tile([SLICEP, 128], BF16)
        hext_tab = dram.tile([NTAB, 128], BF16)
        h2_tab = dram.tile([NTAB, 128], BF16)
        hext_rep = dram.tile([NC, SLICEP, 128], BF16)
        h2_rep = dram.tile([NC, SLICEP, 128], BF16)

        cpool = top.enter_context(tc.tile_pool(name="consts", bufs=1))
        w1t = cpool.tile([128, KT, F], BF16)
        nc.sync.dma_start(out=w1t[:], in_=w1t_d[:])
        adst = cpool.tile([F, H], BF16)
        nc.sync.dma_start(out=adst[:], in_=adst_d[:])
        arep = cpool.tile([128, F], BF16)
        nc.sync.dma_start(out=arep[:], in_=arep_d[:])
        w2e = cpool.tile([F, C + 2], BF16)
        nc.sync.dma_start(out=w2e[:], in_=w2e_d[:])
        rconst = cpool.tile([128, 128], BF16)
        nc.sync.dma_start(out=rconst[:], in_=rconst_d[:])
        ident = cpool.tile([128, 128], BF16)
        nc.sync.dma_start(out=ident[:], in_=ident_d[:])
        b1rep = cpool.tile([128, F], F32)
        nc.sync.dma_start(out=b1rep[:], in_=b1rep_d[:])
        b2rep = cpool.tile([128, C], F32)
        nc.sync.dma_start(out=b2rep[:], in_=b2rep_d[:])
        src16 = cpool.tile([128, NBB * 8], I16)
        nc.sync.dma_start(out=src16[:], in_=src16_d[:])
        adc16 = cpool.tile([128, NBB * 8], I16)
        nc.sync.dma_start(out=adc16[:], in_=adc16_d[:])
        dsl = cpool.tile([128, NBB], BF16)
        nc.sync.dma_start(out=dsl[:], in_=dsl_d[:])
        par = cpool.tile([128, NBB], BF16)
        nc.sync.dma_start(out=par[:], in_=par_d[:])
        parn = cpool.tile([128, NBB], BF16)
        nc.sync.dma_start(out=parn[:], in_=parn_d[:])

        # ---------------- phase 0: node table build ----------------
        with ExitStack() as ph0:
            sb = ph0.enter_context(tc.tile_pool(name="p0sb", bufs=2))
            ps = ph0.enter_context(tc.tile_pool(name="p0ps", bufs=2, space="PSUM"))
            NJ = TILE_N // 128
            for t in range(T0):
                xt = sb.tile([128, KT, TILE_N], BF16, tag="xt")
                nc.sync.dma_start(
                    out=xt[:],
                    in_=xt_d[:, t * TILE_N:(t + 1) * TILE_N]
                    .rearrange("(i p) n -> p i n", p=128))
                psum_h = ps.tile([128, TILE_N], F32, tag="ph")
                for i in range(KT):
                    nc.tensor.matmul(psum_h[:], lhsT=w1t[:, i, :], rhs=xt[:, i, :],
                                     start=(i == 0), stop=(i == KT - 1))
                hsb = sb.tile([F, TILE_N], BF16, tag="hsb")
                nc.vector.tensor_copy(out=hsb[:], in_=psum_h[:F, :])
                psum_aa = ps.tile([H, TILE_N], F32, tag="paa")
                nc.tensor.matmul(psum_aa[:], lhsT=adst[:], rhs=hsb[:],
                                 start=True, stop=True)
                aasb = sb.tile([H, TILE_N], BF16, tag="aasb")
                nc.vector.tensor_copy(out=aasb[:], in_=psum_aa[:])
                psum_hT = ps.tile([128, NJ, 128], BF16, tag="pht")
                for j in range(NJ):
                    nc.tensor.transpose(out=psum_hT[:, j, :F],
                                        in_=hsb[:, j * 128:(j + 1) * 128],
                                        identity=ident[:])
                psum_aaT = ps.tile([128, NJ, H], BF16, tag="paat")
                for j in range(NJ):
                    nc.tensor.transpose(out=psum_aaT[:, j, :],
                                        in_=aasb[:, j * 128:(j + 1) * 128],
                                        identity=ident[:H, :H])
                hx = sb.tile([128, NJ, 128], BF16, tag="hx")
                nc.vector.tensor_copy(out=hx[:], in_=psum_hT[:])
                nc.sync.dma_start(
                    out=hext_loc[t * TILE_N:(t + 1) * TILE_N, :]
                    .rearrange("(j p) c -> p j c", p=128),
                    in_=hx[:])
                adcsb = sb.tile([128, NJ, H], BF16, tag="adcsb")
                nc.vector.tensor_copy(out=adcsb[:], in_=psum_aaT[:])
                nc.sync.dma_start(
                    out=adc1_t[t * TILE_N:(t + 1) * TILE_N, 0:H]
                    .rearrange("(j p) c -> p j c", p=128),
                    in_=adcsb[:])

        # sentinel rows (finite h, -200 adc), then all-gather
        nc.sync.dma_start(out=hext_loc[SLICE:SLICE + 1, :], in_=zrow_d[:])
        nc.sync.dma_start(out=adc1_t[SLICE:SLICE + 1, 0:H], in_=sent_d[:, 0:H])
        nc.sync.dma_start(out=adc2_t[SLICE:SLICE + 1, 0:1], in_=sent_d[:, 0:1])
        if NC == 1:
            nc.sync.dma_start(out=hext_tab[0:SLICEP, :], in_=hext_loc[:])
        else:
            for j in range(NC):
                nc.sync.dma_start(out=hext_rep[j], in_=hext_loc[:])
            nc.gpsimd.collective_compute(
                "AllToAll", OP.bypass, replica_groups=rg,
                ins=[hext_rep[:].rearrange("c r f -> c (r f)").opt()],
                outs=[hext_tab[0:NC * SLICEP, :]
                      .rearrange("(c r) f -> c (r f)", c=NC).opt()])

        hext_pair = hext_tab[:].rearrange("(n two) c -> n (two c)", two=2)
        h2_pair = h2_tab[:].rearrange("(n two) c -> n (two c)", two=2)

        # ---------------- layer-1 edge phase ----------------
        with ExitStack() as ph1:
            sb = ph1.enter_context(tc.tile_pool(name="l1sb", bufs=2))
            gp = ph1.enter_context(tc.tile_pool(name="l1g", bufs=3))
            ps = ph1.enter_context(tc.tile_pool(name="l1ps", bufs=2, space="PSUM"))
            ps2 = ph1.enter_context(tc.tile_pool(name="l1ps2", bufs=2, space="PSUM"))
            drsb = ph1.enter_context(tc.tile_pool(name="l1dr", bufs=2))
            for ch in range(NCH):
                db0 = ch * CHUNK_DB
                CB = min(CHUNK_DB, NB - db0)
                c0, c1 = db0 * B, (db0 + CB) * B
                CT = CB * B
                NIX = CT * 128
                g2 = gp.tile([128, CTMAX, 256], BF16, tag="g2")
                gadc = gp.tile([128, CTMAX, 128], BF16, tag="gadc")
                T3 = CT // 3
                cuts = [0, T3, 2 * T3, CT]
                for i in range(3):
                    a, b = c0 + cuts[i], c0 + cuts[i + 1]
                    n = (b - a) * 128
                    q = 1 + (ch + i + 1) % 3
                    nc.gpsimd.dma_gather(
                        gadc[:, cuts[i]:cuts[i + 1], :], adc1_t[:],
                        adc16[:, a * 8:b * 8], n, n, 128,
                        single_packet=False, queue_num=q)
                for i in range(3):
                    a, b = c0 + cuts[i], c0 + cuts[i + 1]
                    n = (b - a) * 128
                    q = 1 + (ch + i) % 3
                    nc.gpsimd.dma_gather(
                        g2[:, cuts[i]:cuts[i + 1], :], hext_pair,
                        src16[:, a * 8:b * 8], n, n, 256,
                        single_packet=False, queue_num=q)
                # S one-hot (gather-independent; keeps DVE busy during gathers)
                S = sb.tile([128, CTMAX, 128], BF16, tag="S")
                nc.vector.tensor_tensor(
                    out=S[:, 0:CT, :],
                    in0=dsl[:, c0:c1].unsqueeze(2).to_broadcast([128, CT, 128]),
                    in1=rconst[:].unsqueeze(1).to_broadcast([128, CT, 128]),
                    op=OP.is_equal)
                # parity select: he = lo*parn + hi*par
                he = sb.tile([128, CTMAX, 128], BF16, tag="he")
                tmp = sb.tile([128, CTMAX, 128], BF16, tag="tmp")
                nc.vector.tensor_tensor(
                    out=he[:, 0:CT, :], in0=g2[:, 0:CT, 0:128],
                    in1=parn[:, c0:c1].unsqueeze(2).to_broadcast([128, CT, 128]),
                    op=OP.mult)
                nc.vector.tensor_tensor(
                    out=tmp[:, 0:CT, :], in0=g2[:, 0:CT, 128:256],
                    in1=par[:, c0:c1].unsqueeze(2).to_broadcast([128, CT, 128]),
                    op=OP.mult)
                nc.vector.tensor_add(out=he[:, 0:CT, :], in0=he[:, 0:CT, :],
                                     in1=tmp[:, 0:CT, :])
                # asc = per-head dot(h, a_src)
                nc.vector.tensor_tensor(
                    out=tmp[:, 0:CT, :], in0=he[:, 0:CT, :],
                    in1=arep[:].unsqueeze(1).to_broadcast([128, CT, 128]),
                    op=OP.mult)
                asc = sb.tile([128, CTMAX * H, 1], F32, tag="asc")
                nc.vector.tensor_reduce(
                    out=asc[:, 0:CT * H, :],
                    in_=tmp[:, 0:CT, :].rearrange("p j (h c) -> p (j h) c", c=cfg.HID),
                    axis=AX.X, op=OP.add)
                # p = exp(lrelu(asc+adc))
                ee = sb.tile([128, CTMAX, H], F32, tag="ee")
                nc.vector.tensor_tensor(
                    out=ee[:, 0:CT, :],
                    in0=asc[:, 0:CT * H, :].rearrange("p (j h) o -> p j (h o)", h=H),
                    in1=gadc[:, 0:CT, 0:H], op=OP.add)
                nc.vector.scalar_tensor_tensor(
                    out=ee[:, 0:CT, :], in0=ee[:, 0:CT, :], scalar=0.2,
                    in1=ee[:, 0:CT, :], op0=OP.mult, op1=OP.max)
                pt = sb.tile([128, CTMAX, H], BF16, tag="pt")
                nc.scalar.activation(pt[:, 0:CT, :], ee[:, 0:CT, :], AF.Exp)
                # rhs = [h*p | p]
                rhs = sb.tile([128, CTMAX, FR], BF16, tag="rhs")
                nc.vector.tensor_tensor(
                    out=rhs[:, 0:CT, 0:F].rearrange("p j (h c) -> p j h c", c=cfg.HID),
                    in0=he[:, 0:CT, :].rearrange("p j (h c) -> p j h c", c=cfg.HID),
                    in1=pt[:, 0:CT, :].unsqueeze(3).to_broadcast([128, CT, H, cfg.HID]),
                    op=OP.mult)
                nc.vector.tensor_copy(out=rhs[:, 0:CT, F:FR], in_=pt[:, 0:CT, :])

                for lb in range(CB):
                    db = db0 + lb
                    acc = ps.tile([128, FR], F32, tag="acc")
                    for j in range(B):
                        jj = lb * B + j
                        nc.tensor.matmul(acc[:], lhsT=S[:, jj, :],
                                         rhs=rhs[:, jj, :],
                                         start=(j == 0), stop=(j == B - 1))
                    rec = drsb.tile([128, H], F32, tag="rec")
                    nc.vector.tensor_scalar_add(rec[:], acc[:, F:FR], 1e-16)
                    nc.vector.reciprocal(rec[:], rec[:])
                    o1 = drsb.tile([128, F], F32, tag="o1")
                    nc.vector.tensor_tensor(
                        out=o1[:].rearrange("p (h c) -> p h c", c=cfg.HID),
                        in0=acc[:, 0:F].rearrange("p (h c) -> p h c", c=cfg.HID),
                        in1=rec[:].unsqueeze(2).to_broadcast([128, H, cfg.HID]),
                        op=OP.mult)
                    nc.vector.tensor_add(out=o1[:], in0=o1[:], in1=b1rep[:])
                    r1 = drsb.tile([128, F], BF16, tag="r1")
                    nc.scalar.activation(r1[:], o1[:], AF.Relu)
                    pt1 = ps2.tile([128, 128], BF16, tag="pt1")
                    nc.tensor.transpose(out=pt1[:, :F], in_=r1[:], identity=ident[:])
                    r1T = drsb.tile([F, 128], BF16, tag="r1T")
                    nc.vector.tensor_copy(out=r1T[:], in_=pt1[:F, :])
                    ph2 = ps2.tile([128, C + 2], F32, tag="ph2")
                    nc.tensor.matmul(ph2[:], lhsT=r1T[:], rhs=w2e[:],
                                     start=True, stop=True)
                    h2x = drsb.tile([128, C + 1], BF16, tag="h2x")
                    nc.vector.tensor_copy(out=h2x[:], in_=ph2[:, 0:C + 1])
                    nv = 128 if db < NB - 1 else LASTV
                    nc.sync.dma_start(
                        out=h2_loc[db * 128:db * 128 + nv, 0:C + 1],
                        in_=h2x[:nv, :])
                    a2x = drsb.tile([128, 1], BF16, tag="a2x")
                    nc.vector.tensor_copy(out=a2x[:], in_=ph2[:, C + 1:C + 2])
                    nc.sync.dma_start(
                        out=adc2_t[db * 128:db * 128 + nv, 0:1], in_=a2x[:nv, :])

        # sentinel + zero-pad cols, then all-gather layer-2 table
        nc.sync.dma_start(out=h2_loc[SLICE:SLICE + 1, :], in_=zrow_d[:])
        if NC == 1:
            nc.sync.dma_start(out=h2_tab[0:SLICEP, :], in_=h2_loc[:])
        else:
            for j in range(NC):
                nc.sync.dma_start(out=h2_rep[j], in_=h2_loc[:])
            nc.gpsimd.collective_compute(
                "AllToAll", OP.bypass, replica_groups=rg,
                ins=[h2_rep[:].rearrange("c r f -> c (r f)").opt()],
                outs=[h2_tab[0:NC * SLICEP, :]
                      .rearrange("(c r) f -> c (r f)", c=NC).opt()])

        # ---------------- layer-2 edge phase ----------------
        with ExitStack() as ph2s:
            sb = ph2s.enter_context(tc.tile_pool(name="l2sb", bufs=2))
            gp = ph2s.enter_context(tc.tile_pool(name="l2g", bufs=3))
            ps = ph2s.enter_context(tc.tile_pool(name="l2ps", bufs=2, space="PSUM"))
            drsb = ph2s.enter_context(tc.tile_pool(name="l2dr", bufs=2))
            for ch in range(NCH):
                db0 = ch * CHUNK_DB
                CB = min(CHUNK_DB, NB - db0)
                c0, c1 = db0 * B, (db0 + CB) * B
                CT = CB * B
                NIX = CT * 128
                g2 = gp.tile([128, CTMAX, 256], BF16, tag="g2b")
                gadc = gp.tile([128, CTMAX, 128], BF16, tag="gadc2")
                T3 = CT // 3
                cuts = [0, T3, 2 * T3, CT]
                for i in range(3):
                    a, b = c0 + cuts[i], c0 + cuts[i + 1]
                    n = (b - a) * 128
                    q = 1 + (ch + i + 1) % 3
                    nc.gpsimd.dma_gather(
                        gadc[:, cuts[i]:cuts[i + 1], :], adc2_t[:],
                        adc16[:, a * 8:b * 8], n, n, 128,
                        single_packet=False, queue_num=q)
                for i in range(3):
                    a, b = c0 + cuts[i], c0 + cuts[i + 1]
                    n = (b - a) * 128
                    q = 1 + (ch + i) % 3
                    nc.gpsimd.dma_gather(
                        g2[:, cuts[i]:cuts[i + 1], :], h2_pair,
                        src16[:, a * 8:b * 8], n, n, 256,
                        single_packet=False, queue_num=q)
                S = sb.tile([128, CTMAX, 128], BF16, tag="S2")
                nc.vector.tensor_tensor(
                    out=S[:, 0:CT, :],
                    in0=dsl[:, c0:c1].unsqueeze(2).to_broadcast([128, CT, 128]),
                    in1=rconst[:].unsqueeze(1).to_broadcast([128, CT, 128]),
                    op=OP.is_equal)
                he = sb.tile([128, CTMAX, C + 1], BF16, tag="he2")
                tmp = sb.tile([128, CTMAX, C + 1], BF16, tag="tmp2")
                nc.vector.tensor_tensor(
                    out=he[:, 0:CT, :], in0=g2[:, 0:CT, 0:C + 1],
                    in1=parn[:, c0:c1].unsqueeze(2).to_broadcast([128, CT, C + 1]),
                    op=OP.mult)
                nc.vector.tensor_tensor(
                    out=tmp[:, 0:CT, :], in0=g2[:, 0:CT, 128:128 + C + 1],
                    in1=par[:, c0:c1].unsqueeze(2).to_broadcast([128, CT, C + 1]),
                    op=OP.mult)
                nc.vector.tensor_add(out=he[:, 0:CT, :], in0=he[:, 0:CT, :],
                                     in1=tmp[:, 0:CT, :])
                ee = sb.tile([128, CTMAX, 1], F32, tag="ee2")
                nc.vector.tensor_tensor(out=ee[:, 0:CT, :],
                                        in0=he[:, 0:CT, C:C + 1],
                                        in1=gadc[:, 0:CT, 0:1], op=OP.add)
                nc.vector.scalar_tensor_tensor(
                    out=ee[:, 0:CT, :], in0=ee[:, 0:CT, :], scalar=0.2,
                    in1=ee[:, 0:CT, :], op0=OP.mult, op1=OP.max)
                pt = sb.tile([128, CTMAX, 1], BF16, tag="pt2")
                nc.scalar.activation(pt[:, 0:CT, :], ee[:, 0:CT, :], AF.Exp)
                rhs = sb.tile([128, CTMAX, FR2], BF16, tag="rhs2")
                nc.vector.tensor_tensor(
                    out=rhs[:, 0:CT, 0:C],
                    in0=he[:, 0:CT, 0:C],
                    in1=pt[:, 0:CT, :].to_broadcast([128, CT, C]),
                    op=OP.mult)
                nc.vector.tensor_copy(out=rhs[:, 0:CT, C:FR2], in_=pt[:, 0:CT, :])

                for lb in range(CB):
                    db = db0 + lb
                    acc = ps.tile([128, FR2], F32, tag="acc2")
                    for j in range(B):
                        jj = lb * B + j
                        nc.tensor.matmul(acc[:], lhsT=S[:, jj, :],
                                         rhs=rhs[:, jj, :],
                                         start=(j == 0), stop=(j == B - 1))
                    rec = drsb.tile([128, 1], F32, tag="rec2")
                    nc.vector.tensor_scalar_add(rec[:], acc[:, C:FR2], 1e-16)
                    nc.vector.reciprocal(rec[:], rec[:])
                    o2 = drsb.tile([128, C], F32, tag="o2")
                    nc.vector.tensor_tensor(
                        out=o2[:], in0=acc[:, 0:C],
                        in1=rec[:].to_broadcast([128, C]), op=OP.mult)
                    nc.vector.tensor_add(out=o2[:], in0=o2[:], in1=b2rep[:])
                    mneg = drsb.tile([128, 1], F32, tag="mneg")
                    nc.vector.tensor_reduce(out=mneg[:], in_=o2[:], axis=AX.X,
                                            op=OP.max, negate=True)
                    escr = drsb.tile([128, C], F32, tag="escr")
                    ssum = drsb.tile([128, 1], F32, tag="ssum")
                    nc.scalar.activation(escr[:], o2[:], AF.Exp,
                                         bias=mneg[:, 0:1], accum_out=ssum[:])
                    lns = drsb.tile([128, 1], F32, tag="lns")
                    nc.scalar.activation(lns[:], ssum[:], AF.Ln)
                    tsh = drsb.tile([128, 1], F32, tag="tsh")
                    nc.vector.tensor_sub(out=tsh[:], in0=mneg[:], in1=lns[:])
                    fin = drsb.tile([128, C], F32, tag="fin")
                    nc.vector.tensor_tensor(out=fin[:], in0=o2[:],
                                            in1=tsh[:].to_broadcast([128, C]),
                                            op=OP.add)
                    nv = 128 if db < NB - 1 else LASTV
                    nc.sync.dma_start(out=out_d[db * 128:db * 128 + nv, :],
                                      in_=fin[:nv, :])

    nc.compile()
    return nc


_NC_CACHE: dict = {}


def _get_nc(cfg: Cfg):
    if cfg not in _NC_CACHE:
        _NC_CACHE[cfg] = build_nc(cfg)
    return _NC_CACHE[cfg]


def kernel(x, edge_index, W1, a_src1, a_dst1, b1, W2, a_src2, a_dst2, b2,
           cfg: Cfg | None = None, _run=None):
    x = np.asarray(x)
    edge_index = np.asarray(edge_index)
    if cfg is None:
        cfg = Cfg()
        loops = np.arange(cfg.N, dtype=np.int64)
        dst = np.concatenate([np.asarray(edge_index[1]).astype(np.int64), loops])
        b = compute_B(dst, cfg)
        if b != cfg.B:
            cfg = Cfg(B=b)
    in_maps = build_in_maps(x, edge_index, W1, a_src1, a_dst1, b1,
                            W2, a_src2, a_dst2, b2, cfg)
    nc = _get_nc(cfg)
    if _run is not None:
        results = _run(nc, in_maps)
    else:
        res = run_bass_kernel_spmd(nc, in_maps, list(range(cfg.NC)))
        results = res.results
    out = np.concatenate([results[c]["out"] for c in range(cfg.NC)], axis=0)
    return out.astype(np.float32)


# revision 9
# speedup vs baseline: 1.0557x; 1.0557x over previous
"""GAT 2-layer network (PyG GATConv x2 + log_softmax) on 8 Trainium2 NeuronCores.

Strategy (dst-sharded message passing, SWDGE dma_gather edition):
  - Nodes are 1D-sharded across the 8 cores (6250 nodes/core). Edges (with
    self-loops) are assigned to the core owning their *destination* node.
  - Phase 0 (per core): h = x @ W1.T for the core's node slice (rows of
    exactly 128 bf16 = 256B) written to a DRAM table; per-node adc = h.a_dst
    written to a local [SLICEP, 128] table (cols 0:4).
  - AllGather (via AllToAll) the h table so every core has all nodes' rows.
  - Edge phase (per core): edges sorted by dst, grouped into 128-dst-node
    "dst-blocks", each padded to B 128-edge columns.  Per chunk of dst-blocks:
      * ONE custom dma_gather fetches 512B row-PAIRS by src//2 (int16-safe
        indices), a DVE select by src parity extracts the right 256B row.
      * ONE dma_gather fetches per-edge adc rows from the local table (local
        dst index, int16-safe).  Gathers rotate over SWDGE queues 1..3 so
        descriptor generation runs on three Q7 core-pairs in parallel.
      * per-edge asc = sum(h * a_src) via DVE mult+grouped-reduce.
      * p = exp(leaky_relu(asc_src + adc_dst)); padding edges point at a
        -200 adc sentinel row so p ~ 0.
      * one-hot selection matrix S[e, d] = (dstslot[e] == d) built on DVE;
        PSUM-accumulated matmuls S.T @ [h*p | p] give per-dst sums.
      * drain: out = (sum p h)/denom + b, relu -> layer-2 rows
        [h2 | asc2] and local adc2, written to layer-2 tables.
  - AllToAll the layer-2 table, run the same edge phase with 32 features and
    1 head, finish with log_softmax per node, write the output slice.

The kernel() entry point takes the FULL inputs and returns the FULL output.
"""

import math
from contextlib import ExitStack
from dataclasses import dataclass

import numpy as np

import concourse.bass as bass
import concourse.mybir as mybir
import concourse.tile as tile
from concourse import bacc
from concourse.bass_utils import run_bass_kernel_spmd

F32 = mybir.dt.float32
BF16 = mybir.dt.bfloat16
I16 = mybir.dt.int16
AX = mybir.AxisListType
OP = mybir.AluOpType
AF = mybir.ActivationFunctionType

SENT_ADC = -200.0  # sentinel adc: exp(lrelu(asc-200)) ~ 0


@dataclass(frozen=True)
class Cfg:
    N: int = 50000
    F_IN: int = 256
    HID: int = 32
    HEADS: int = 4
    CLASSES: int = 32
    NC: int = 8
    TILE_N: int = 512   # phase-0 node tile
    CHUNK_DB: int = 2   # dst-blocks per gather chunk
    B: int = 18         # uniform 128-edge columns per dst-block (data dep.)

    @property
    def F(self):  # layer-1 feature width == 128
        return self.HID * self.HEADS

    @property
    def NPC(self):
        return self.N // self.NC

    @property
    def T0(self):
        return math.ceil(self.NPC / self.TILE_N)

    @property
    def SLICE(self):
        return self.T0 * self.TILE_N

    @property
    def NB(self):  # dst-blocks per core
        return math.ceil(self.NPC / 128)

    @property
    def SLICEP(self):  # per-core table slice incl sentinel row
        return self.SLICE + 1

    @property
    def NTAB(self):  # node-table rows (must be even for pair-gather)
        n = self.NC * self.SLICEP
        return n + (n % 2)

    @property
    def NCH(self):
        return math.ceil(self.NB / self.CHUNK_DB)

    @property
    def NBB(self):
        return self.NB * self.B

    @property
    def FR(self):  # layer-1 rhs width: h*p(F) + p(HEADS)
        return self.F + self.HEADS

    @property
    def FR2(self):  # layer-2 rhs width
        return self.CLASSES + 1


def compute_B(dst, cfg: Cfg):
    """Max 128-edge columns needed by any (core, dst-block)."""
    NPC, NB = cfg.NPC, cfg.NB
    core = dst // NPC
    blk = (dst % NPC) // 128
    cnt = np.bincount(core * NB + blk, minlength=cfg.NC * NB)
    return int(np.max((cnt + 127) // 128))


def build_layout(src, dst, cfg: Cfg):
    """Per-core edge arrays.

    Edge stream order within a chunk: t = col*128 + p (the dma_gather output
    row order).  Returns per-core:
      src16 [128, NBB*8] i16 : (src_coord >> 1), wrapped 16-partition layout
      adc16 [128, NBB*8] i16 : local dst row (or sentinel), wrapped
      dsl   [128, NBB]  f32 : dst slot within dst-block (0..127)
      par/parn [128, NBB] f32 : src parity and 1-parity
    """
    NC, NPC, NB, B = cfg.NC, cfg.NPC, cfg.NB, cfg.B
    SLICEP = cfg.SLICEP
    NBB = NB * B

    core = dst // NPC
    local = dst % NPC
    blk = local // 128
    seg = core * NB + blk
    order = np.argsort(seg, kind="stable")
    seg_s = seg[order]
    src_s = src[order]
    local_s = local[order]

    cnt = np.bincount(seg, minlength=NC * NB)
    seg_start = np.zeros(NC * NB + 1, dtype=np.int64)
    np.cumsum(cnt, out=seg_start[1:])
    pos = np.arange(len(dst)) - seg_start[seg_s]
    assert pos.max() < B * 128, "B too small for this edge distribution"

    c_s = seg_s // NB
    blk_s = seg_s % NB
    p_s = (pos % 128).astype(np.int64)
    col_s = (blk_s * B + pos // 128).astype(np.int64)

    src_core = src_s // NPC
    src_local = src_s % NPC
    src_coord = (src_core * SLICEP + src_local).astype(np.int64)

    # dense per-core arrays; pads: src_coord=0, adc->sentinel(SLICE), dsl=0
    srcc = np.zeros((NC, 128, NBB), dtype=np.int64)
    adci = np.full((NC, 128, NBB), cfg.SLICE, dtype=np.int64)
    dsl = np.zeros((NC, 128, NBB), dtype=np.float32)
    srcc[c_s, p_s, col_s] = src_coord
    adci[c_s, p_s, col_s] = local_s
    dsl[c_s, p_s, col_s] = (local_s - blk_s * 128).astype(np.float32)

    par = (srcc & 1).astype(np.float32)
    src16 = (srcc >> 1).astype(np.int16)
    adc16 = adci.astype(np.int16)

    # wrap into the dma_gather idx layout: stream t = col*128 + p;
    # wrapped[t % 16, t // 16], replicated to all 128 partitions.  Chunks
    # start at column (=128-edge) multiples so per-chunk slices stay aligned.
    p_idx = np.arange(128)
    c_idx = np.arange(NBB)
    t = (c_idx[None, :] * 128 + p_idx[:, None])        # [128, NBB]
    wp = (t % 16).ravel()
    wf = (t // 16).ravel()

    def wrap_fast(a16):
        out = np.zeros((NC, 16, NBB * 8), dtype=np.int16)
        flat = a16.reshape(NC, 128 * NBB)
        out[:, wp, wf] = flat
        return np.tile(out, (1, 8, 1))

    return wrap_fast(src16), wrap_fast(adc16), dsl, par, 1.0 - par


def pack_consts(W1, a_src1, a_dst1, b1, W2, a_src2, a_dst2, b2, cfg: Cfg):
    F, H, HID, C, F_IN = cfg.F, cfg.HEADS, cfg.HID, cfg.CLASSES, cfg.F_IN
    KT = F_IN // 128
    consts = {}
    w1t = W1.T.reshape(KT, 128, F).transpose(1, 0, 2)
    consts["w1t"] = w1t.astype(np.float32)
    # a_dst blockdiag [F, H]
    adst = np.zeros((F, H), dtype=np.float32)
    for h in range(H):
        adst[h * HID:(h + 1) * HID, h] = a_dst1[h]
    consts["adst"] = adst
    # a_src replicated [128, F]
    consts["arep"] = np.tile(a_src1.reshape(1, F), (128, 1)).astype(np.float32)
    # W2ext [F, C+2]: W2.T | (a_src2@W2).T | (a_dst2@W2).T
    w2e = np.concatenate(
        [W2.T.astype(np.float64),
         (a_src2 @ W2).T.astype(np.float64),
         (a_dst2 @ W2).T.astype(np.float64)], axis=1)
    consts["w2e"] = w2e.astype(np.float32)
    consts["rconst"] = np.tile(np.arange(128, dtype=np.float32)[None, :], (128, 1))
    consts["ident"] = np.eye(128, dtype=np.float32)
    consts["b1rep"] = np.tile(b1[None, :].astype(np.float32), (128, 1))
    consts["b2rep"] = np.tile(b2[None, :].astype(np.float32), (128, 1))
    consts["sent"] = np.full((1, 128), SENT_ADC, dtype=np.float32)
    consts["zrow"] = np.zeros((1, 128), dtype=np.float32)
    return consts


def build_in_maps(x, edge_index, W1, a_src1, a_dst1, b1, W2, a_src2, a_dst2, b2,
                  cfg: Cfg):
    N, NC, NPC, SLICE = cfg.N, cfg.NC, cfg.NPC, cfg.SLICE
    loops = np.arange(N, dtype=edge_index.dtype)
    src = np.concatenate([np.asarray(edge_index[0]), loops]).astype(np.int64)
    dst = np.concatenate([np.asarray(edge_index[1]), loops]).astype(np.int64)

    src16, adc16, dsl, par, parn = build_layout(src, dst, cfg)
    consts = pack_consts(np.asarray(W1), np.asarray(a_src1), np.asarray(a_dst1),
                         np.asarray(b1), np.asarray(W2), np.asarray(a_src2),
                         np.asarray(a_dst2), np.asarray(b2), cfg)

    xT = np.zeros((cfg.F_IN, NC * SLICE), dtype=np.float32)
    xv = np.asarray(x).T
    for c in range(NC):
        xT[:, c * SLICE:c * SLICE + NPC] = xv[:, c * NPC:(c + 1) * NPC]

    import ml_dtypes
    to_bf16 = lambda a: a.astype(ml_dtypes.bfloat16)

    in_maps = []
    for c in range(NC):
        m = {
            "xt": to_bf16(xT[:, c * SLICE:(c + 1) * SLICE]),
            "src16": src16[c],
            "adc16": adc16[c],
            "dsl": to_bf16(dsl[c]),
            "par": to_bf16(par[c]),
            "parn": to_bf16(parn[c]),
            "w1t": to_bf16(consts["w1t"]),
            "adst": to_bf16(consts["adst"]),
            "arep": to_bf16(consts["arep"]),
            "w2e": to_bf16(consts["w2e"]),
            "rconst": to_bf16(consts["rconst"]),
            "ident": to_bf16(consts["ident"]),
            "b1rep": consts["b1rep"],
            "b2rep": consts["b2rep"],
            "sent": to_bf16(consts["sent"]),
            "zrow": to_bf16(consts["zrow"]),
        }
        in_maps.append(m)
    return in_maps


def build_nc(cfg: Cfg):
    """Build + compile the SPMD program (identical for all cores)."""
    N, NC, NPC, NB, B, SLICE, T0 = (cfg.N, cfg.NC, cfg.NPC, cfg.NB, cfg.B,
                                    cfg.SLICE, cfg.T0)
    F, H, C, F_IN = cfg.F, cfg.HEADS, cfg.CLASSES, cfg.F_IN
    FR, FR2 = cfg.FR, cfg.FR2
    TILE_N, CHUNK_DB, NCH = cfg.TILE_N, cfg.CHUNK_DB, cfg.NCH
    NBB = NB * B
    KT = F_IN // 128
    SLICEP = cfg.SLICEP
    NTAB = cfg.NTAB
    LASTV = NPC - (NB - 1) * 128
    CTMAX = CHUNK_DB * B

    nc = bacc.Bacc("TRN2", target_bir_lowering=False, debug=False,
                   num_devices=NC, num_swdge_queues=4)

    dt = nc.dram_tensor
    xt_d = dt("xt", [F_IN, SLICE], BF16, kind="ExternalInput").ap()
    src16_d = dt("src16", [128, NBB * 8], I16, kind="ExternalInput").ap()
    adc16_d = dt("adc16", [128, NBB * 8], I16, kind="ExternalInput").ap()
    dsl_d = dt("dsl", [128, NBB], BF16, kind="ExternalInput").ap()
    par_d = dt("par", [128, NBB], BF16, kind="ExternalInput").ap()
    parn_d = dt("parn", [128, NBB], BF16, kind="ExternalInput").ap()
    w1t_d = dt("w1t", [128, KT, F], BF16, kind="ExternalInput").ap()
    adst_d = dt("adst", [F, H], BF16, kind="ExternalInput").ap()
    arep_d = dt("arep", [128, F], BF16, kind="ExternalInput").ap()
    w2e_d = dt("w2e", [F, C + 2], BF16, kind="ExternalInput").ap()
    rconst_d = dt("rconst", [128, 128], BF16, kind="ExternalInput").ap()
    ident_d = dt("ident", [128, 128], BF16, kind="ExternalInput").ap()
    b1rep_d = dt("b1rep", [128, F], F32, kind="ExternalInput").ap()
    b2rep_d = dt("b2rep", [128, C], F32, kind="ExternalInput").ap()
    sent_d = dt("sent", [1, 128], BF16, kind="ExternalInput").ap()
    zrow_d = dt("zrow", [1, 128], BF16, kind="ExternalInput").ap()
    out_d = dt("out", [NPC, C], F32, kind="ExternalOutput").ap()

    rg = [list(range(NC))]

    with tile.TileContext(nc) as tc, ExitStack() as top:
        dram = top.enter_context(tc.tile_pool(name="dram", bufs=1, space="DRAM"))
        hext_loc = dram.tile([SLICEP, 128], BF16)
        h2_loc = dram.tile([SLICEP, 128], BF16)
        adc1_t = dram.tile([SLICEP, 128], BF16)
        adc2_t = dram.tile([SLICEP, 128], BF16)
        hext_tab = dram.tile([NTAB, 128], BF16)
        h2_tab = dram.tile([NTAB, 128], BF16)
        hext_rep = dram.tile([NC, SLICEP, 128], BF16)
        h2_rep = dram.tile([NC, SLICEP, 128], BF16)

        cpool = top.enter_context(tc.tile_pool(name="consts", bufs=1))
        w1t = cpool.tile([128, KT, F], BF16)
        nc.sync.dma_start(out=w1t[:], in_=w1t_d[:])
        adst = cpool.tile([F, H], BF16)
        nc.sync.dma_start(out=adst[:], in_=adst_d[:])
        arep = cpool.tile([128, F], BF16)
        nc.sync.dma_start(out=arep[:], in_=arep_d[:])
        w2e = cpool.tile([F, C + 2], BF16)
        nc.sync.dma_start(out=w2e[:], in_=w2e_d[:])
        rconst = cpool.tile([128, 128], BF16)
        nc.sync.dma_start(out=rconst[:], in_=rconst_d[:])
        ident = cpool.tile([128, 128], BF16)
        nc.sync.dma_start(out=ident[:], in_=ident_d[:])
        b1rep = cpool.tile([128, F], F32)
        nc.sync.dma_start(out=b1rep[:], in_=b1rep_d[:])
        b2rep = cpool.tile([128, C], F32)
        nc.sync.dma_start(out=b2rep[:], in_=b2rep_d[:])
        src16 = cpool.tile([128, NBB * 8], I16)
        nc.sync.dma_start(out=src16[:], in_=src16_d[:])
        adc16 = cpool.tile([128, NBB * 8], I16)
        nc.sync.dma_start(out=adc16[:], in_=adc16_d[:])
        dsl = cpool.tile([128, NBB], BF16)
        nc.sync.dma_start(out=dsl[:], in_=dsl_d[:])
        par = cpool.tile([128, NBB], BF16)
        nc.sync.dma_start(out=par[:], in_=par_d[:])
        parn = cpool.tile([128, NBB], BF16)
        nc.sync.dma_start(out=parn[:], in_=parn_d[:])

        # ---------------- phase 0: node table build ----------------
        with ExitStack() as ph0:
            sb = ph0.enter_context(tc.tile_pool(name="p0sb", bufs=2))
            ps = ph0.enter_context(tc.tile_pool(name="p0ps", bufs=2, space="PSUM"))
            NJ = TILE_N // 128
            for t in range(T0):
                xt = sb.tile([128, KT, TILE_N], BF16, tag="xt")
                nc.sync.dma_start(
                    out=xt[:],
                    in_=xt_d[:, t * TILE_N:(t + 1) * TILE_N]
                    .rearrange("(i p) n -> p i n", p=128))
                psum_h = ps.tile([128, TILE_N], F32, tag="ph")
                for i in range(KT):
                    nc.tensor.matmul(psum_h[:], lhsT=w1t[:, i, :], rhs=xt[:, i, :],
                                     start=(i == 0), stop=(i == KT - 1))
                hsb = sb.tile([F, TILE_N], BF16, tag="hsb")
                nc.vector.tensor_copy(out=hsb[:], in_=psum_h[:F, :])
                psum_aa = ps.tile([H, TILE_N], F32, tag="paa")
                nc.tensor.matmul(psum_aa[:], lhsT=adst[:], rhs=hsb[:],
                                 start=True, stop=True)
                aasb = sb.tile([H, TILE_N], BF16, tag="aasb")
                nc.vector.tensor_copy(out=aasb[:], in_=psum_aa[:])
                psum_hT = ps.tile([128, NJ, 128], BF16, tag="pht")
                for j in range(NJ):
                    nc.tensor.transpose(out=psum_hT[:, j, :F],
                                        in_=hsb[:, j * 128:(j + 1) * 128],
                                        identity=ident[:])
                psum_aaT = ps.tile([128, NJ, H], BF16, tag="paat")
                for j in range(NJ):
                    nc.tensor.transpose(out=psum_aaT[:, j, :],
                                        in_=aasb[:, j * 128:(j + 1) * 128],
                                        identity=ident[:H, :H])
                hx = sb.tile([128, NJ, 128], BF16, tag="hx")
                nc.vector.tensor_copy(out=hx[:], in_=psum_hT[:])
                nc.sync.dma_start(
                    out=hext_loc[t * TILE_N:(t + 1) * TILE_N, :]
                    .rearrange("(j p) c -> p j c", p=128),
                    in_=hx[:])
                adcsb = sb.tile([128, NJ, H], BF16, tag="adcsb")
                nc.vector.tensor_copy(out=adcsb[:], in_=psum_aaT[:])
                nc.sync.dma_start(
                    out=adc1_t[t * TILE_N:(t + 1) * TILE_N, 0:H]
                    .rearrange("(j p) c -> p j c", p=128),
                    in_=adcsb[:])

        # sentinel rows (finite h, -200 adc), then all-gather
        nc.sync.dma_start(out=hext_loc[SLICE:SLICE + 1, :], in_=zrow_d[:])
        nc.sync.dma_start(out=adc1_t[SLICE:SLICE + 1, 0:H], in_=sent_d[:, 0:H])
        nc.sync.dma_start(out=adc2_t[SLICE:SLICE + 1, 0:1], in_=sent_d[:, 0:1])
        if NC == 1:
            nc.sync.dma_start(out=hext_tab[0:SLICEP, :], in_=hext_loc[:])
        else:
            for j in range(NC):
                nc.sync.dma_start(out=hext_rep[j], in_=hext_loc[:])
            nc.gpsimd.collective_compute(
                "AllToAll", OP.bypass, replica_groups=rg,
                ins=[hext_rep[:].rearrange("c r f -> c (r f)").opt()],
                outs=[hext_tab[0:NC * SLICEP, :]
                      .rearrange("(c r) f -> c (r f)", c=NC).opt()])

        hext_pair = hext_tab[:].rearrange("(n two) c -> n (two c)", two=2)
        h2_pair = h2_tab[:].rearrange("(n two) c -> n (two c)", two=2)

        # ---------------- layer-1 edge phase ----------------
        with ExitStack() as ph1:
            sb = ph1.enter_context(tc.tile_pool(name="l1sb", bufs=2))
            ps = ph1.enter_context(tc.tile_pool(name="l1ps", bufs=2, space="PSUM"))
            ps2 = ph1.enter_context(tc.tile_pool(name="l1ps2", bufs=2, space="PSUM"))
            drsb = ph1.enter_context(tc.tile_pool(name="l1dr", bufs=2))
            for ch in range(NCH):
                db0 = ch * CHUNK_DB
                CB = min(CHUNK_DB, NB - db0)
                c0, c1 = db0 * B, (db0 + CB) * B
                CT = CB * B
                NIX = CT * 128
                g2 = sb.tile([128, CTMAX, 256], BF16, tag="g2")
                gadc = sb.tile([128, CTMAX, 128], BF16, tag="gadc")
                T3 = CT // 3
                cuts = [0, T3, 2 * T3, CT]
                for i in range(3):
                    a, b = c0 + cuts[i], c0 + cuts[i + 1]
                    n = (b - a) * 128
                    q = 1 + (ch + i + 1) % 3
                    nc.gpsimd.dma_gather(
                        gadc[:, cuts[i]:cuts[i + 1], :], adc1_t[:],
                        adc16[:, a * 8:b * 8], n, n, 128,
                        single_packet=False, queue_num=q)
                for i in range(3):
                    a, b = c0 + cuts[i], c0 + cuts[i + 1]
                    n = (b - a) * 128
                    q = 1 + (ch + i) % 3
                    nc.gpsimd.dma_gather(
                        g2[:, cuts[i]:cuts[i + 1], :], hext_pair,
                        src16[:, a * 8:b * 8], n, n, 256,
                        single_packet=False, queue_num=q)
                # S one-hot (gather-independent; keeps DVE busy during gathers)
                S = sb.tile([128, CTMAX, 128], BF16, tag="S")
                nc.vector.tensor_tensor(
                    out=S[:, 0:CT, :],
                    in0=dsl[:, c0:c1].unsqueeze(2).to_broadcast([128, CT, 128]),
                    in1=rconst[:].unsqueeze(1).to_broadcast([128, CT, 128]),
                    op=OP.is_equal)
                # parity select: he = lo*parn + hi*par
                he = sb.tile([128, CTMAX, 128], BF16, tag="he")
                tmp = sb.tile([128, CTMAX, 128], BF16, tag="tmp")
                nc.vector.tensor_tensor(
                    out=he[:, 0:CT, :], in0=g2[:, 0:CT, 0:128],
                    in1=parn[:, c0:c1].unsqueeze(2).to_broadcast([128, CT, 128]),
                    op=OP.mult)
                nc.vector.tensor_tensor(
                    out=tmp[:, 0:CT, :], in0=g2[:, 0:CT, 128:256],
                    in1=par[:, c0:c1].unsqueeze(2).to_broadcast([128, CT, 128]),
                    op=OP.mult)
                nc.vector.tensor_add(out=he[:, 0:CT, :], in0=he[:, 0:CT, :],
                                     in1=tmp[:, 0:CT, :])
                # asc = per-head dot(h, a_src)
                nc.vector.tensor_tensor(
                    out=tmp[:, 0:CT, :], in0=he[:, 0:CT, :],
                    in1=arep[:].unsqueeze(1).to_broadcast([128, CT, 128]),
                    op=OP.mult)
                asc = sb.tile([128, CTMAX * H, 1], F32, tag="asc")
                nc.vector.tensor_reduce(
                    out=asc[:, 0:CT * H, :],
                    in_=tmp[:, 0:CT, :].rearrange("p j (h c) -> p (j h) c", c=cfg.HID),
                    axis=AX.X, op=OP.add)
                # p = exp(lrelu(asc+adc))
                ee = sb.tile([128, CTMAX, H], F32, tag="ee")
                nc.vector.tensor_tensor(
                    out=ee[:, 0:CT, :],
                    in0=asc[:, 0:CT * H, :].rearrange("p (j h) o -> p j (h o)", h=H),
                    in1=gadc[:, 0:CT, 0:H], op=OP.add)
                nc.vector.scalar_tensor_tensor(
                    out=ee[:, 0:CT, :], in0=ee[:, 0:CT, :], scalar=0.2,
                    in1=ee[:, 0:CT, :], op0=OP.mult, op1=OP.max)
                pt = sb.tile([128, CTMAX, H], BF16, tag="pt")
                nc.scalar.activation(pt[:, 0:CT, :], ee[:, 0:CT, :], AF.Exp)
                # rhs = [h*p | p]
                rhs = sb.tile([128, CTMAX, FR], BF16, tag="rhs")
                nc.vector.tensor_tensor(
                    out=rhs[:, 0:CT, 0:F].rearrange("p j (h c) -> p j h c", c=cfg.HID),
                    in0=he[:, 0:CT, :].rearrange("p j (h c) -> p j h c", c=cfg.HID),
                    in1=pt[:, 0:CT, :].unsqueeze(3).to_broadcast([128, CT, H, cfg.HID]),
                    op=OP.mult)
                nc.vector.tensor_copy(out=rhs[:, 0:CT, F:FR], in_=pt[:, 0:CT, :])

                for lb in range(CB):
                    db = db0 + lb
                    acc = ps.tile([128, FR], F32, tag="acc")
                    for j in range(B):
                        jj = lb * B + j
                        nc.tensor.matmul(acc[:], lhsT=S[:, jj, :],
                                         rhs=rhs[:, jj, :],
                                         start=(j == 0), stop=(j == B - 1))
                    rec = drsb.tile([128, H], F32, tag="rec")
                    nc.vector.tensor_scalar_add(rec[:], acc[:, F:FR], 1e-16)
                    nc.vector.reciprocal(rec[:], rec[:])
                    o1 = drsb.tile([128, F], F32, tag="o1")
                    nc.vector.tensor_tensor(
                        out=o1[:].rearrange("p (h c) -> p h c", c=cfg.HID),
                        in0=acc[:, 0:F].rearrange("p (h c) -> p h c", c=cfg.HID),
                        in1=rec[:].unsqueeze(2).to_broadcast([128, H, cfg.HID]),
                        op=OP.mult)
                    nc.vector.tensor_add(out=o1[:], in0=o1[:], in1=b1rep[:])
                    r1 = drsb.tile([128, F], BF16, tag="r1")
                    nc.scalar.activation(r1[:], o1[:], AF.Relu)
                    pt1 = ps2.tile([128, 128], BF16, tag="pt1")
                    nc.tensor.transpose(out=pt1[:, :F], in_=r1[:], identity=ident[:])
                    r1T = drsb.tile([F, 128], BF16, tag="r1T")
                    nc.vector.tensor_copy(out=r1T[:], in_=pt1[:F, :])
                    ph2 = ps2.tile([128, C + 2], F32, tag="ph2")
                    nc.tensor.matmul(ph2[:], lhsT=r1T[:], rhs=w2e[:],
                                     start=True, stop=True)
                    h2x = drsb.tile([128, C + 1], BF16, tag="h2x")
                    nc.vector.tensor_copy(out=h2x[:], in_=ph2[:, 0:C + 1])
                    nv = 128 if db < NB - 1 else LASTV
                    nc.sync.dma_start(
                        out=h2_loc[db * 128:db * 128 + nv, 0:C + 1],
                        in_=h2x[:nv, :])
                    a2x = drsb.tile([128, 1], BF16, tag="a2x")
                    nc.vector.tensor_copy(out=a2x[:], in_=ph2[:, C + 1:C + 2])
                    nc.sync.dma_start(
                        out=adc2_t[db * 128:db * 128 + nv, 0:1], in_=a2x[:nv, :])

        # sentinel + zero-pad cols, then all-gather layer-2 table
        nc.sync.dma_start(out=h2_loc[SLICE:SLICE + 1, :], in_=zrow_d[:])
        if NC == 1:
            nc.sync.dma_start(out=h2_tab[0:SLICEP, :], in_=h2_loc[:])
        else:
            for j in range(NC):
                nc.sync.dma_start(out=h2_rep[j], in_=h2_loc[:])
            nc.gpsimd.collective_compute(
                "AllToAll", OP.bypass, replica_groups=rg,
                ins=[h2_rep[:].rearrange("c r f -> c (r f)").opt()],
                outs=[h2_tab[0:NC * SLICEP, :]
                      .rearrange("(c r) f -> c (r f)", c=NC).opt()])

        # ---------------- layer-2 edge phase ----------------
        with ExitStack() as ph2s:
            sb = ph2s.enter_context(tc.tile_pool(name="l2sb", bufs=2))
            ps = ph2s.enter_context(tc.tile_pool(name="l2ps", bufs=2, space="PSUM"))
            drsb = ph2s.enter_context(tc.tile_pool(name="l2dr", bufs=2))
            for ch in range(NCH):
                db0 = ch * CHUNK_DB
                CB = min(CHUNK_DB, NB - db0)
                c0, c1 = db0 * B, (db0 + CB) * B
                CT = CB * B
                NIX = CT * 128
                g2 = sb.tile([128, CTMAX, 256], BF16, tag="g2b")
                gadc = sb.tile([128, CTMAX, 128], BF16, tag="gadc2")
                T3 = CT // 3
                cuts = [0, T3, 2 * T3, CT]
                for i in range(3):
                    a, b = c0 + cuts[i], c0 + cuts[i + 1]
                    n = (b - a) * 128
                    q = 1 + (ch + i + 1) % 3
                    nc.gpsimd.dma_gather(
                        gadc[:, cuts[i]:cuts[i + 1], :], adc2_t[:],
                        adc16[:, a * 8:b * 8], n, n, 128,
                        single_packet=False, queue_num=q)
                for i in range(3):
                    a, b = c0 + cuts[i], c0 + cuts[i + 1]
                    n = (b - a) * 128
                    q = 1 + (ch + i) % 3
                    nc.gpsimd.dma_gather(
                        g2[:, cuts[i]:cuts[i + 1], :], h2_pair,
                        src16[:, a * 8:b * 8], n, n, 256,
                        single_packet=False, queue_num=q)
                S = sb.tile([128, CTMAX, 128], BF16, tag="S2")
                nc.vector.tensor_tensor(
                    out=S[:, 0:CT, :],
                    in0=dsl[:, c0:c1].unsqueeze(2).to_broadcast([128, CT, 128]),
                    in1=rconst[:].unsqueeze(1).to_broadcast([128, CT, 128]),
                    op=OP.is_equal)
                he = sb.tile([128, CTMAX, C + 1], BF16, tag="he2")
                tmp = sb.tile([128, CTMAX, C + 1], BF16, tag="tmp2")
                nc.vector.tensor_tensor(
                    out=he[:, 0:CT, :], in0=g2[:, 0:CT, 0:C + 1],
                    in1=parn[:, c0:c1].unsqueeze(2).to_broadcast([128, CT, C + 1]),
                    op=OP.mult)
                nc.vector.tensor_tensor(
                    out=tmp[:, 0:CT, :], in0=g2[:, 0:CT, 128:128 + C + 1],
                    in1=par[:, c0:c1].unsqueeze(2).to_broadcast([128, CT, C + 1]),
                    op=OP.mult)
                nc.vector.tensor_add(out=he[:, 0:CT, :], in0=he[:, 0:CT, :],
                                     in1=tmp[:, 0:CT, :])
                ee = sb.tile([128, CTMAX, 1], F32, tag="ee2")
                nc.vector.tensor_tensor(out=ee[:, 0:CT, :],
                                        in0=he[:, 0:CT, C:C + 1],
                                        in1=gadc[:, 0:CT, 0:1], op=OP.add)
                nc.vector.scalar_tensor_tensor(
                    out=ee[:, 0:CT, :], in0=ee[:, 0:CT, :], scalar=0.2,
                    in1=ee[:, 0:CT, :], op0=OP.mult, op1=OP.max)
                pt = sb.tile([128, CTMAX, 1], BF16, tag="pt2")
                nc.scalar.activation(pt[:, 0:CT, :], ee[:, 0:CT, :], AF.Exp)
                rhs = sb.tile([128, CTMAX, FR2], BF16, tag="rhs2")
                nc.vector.tensor_tensor(
                    out=rhs[:, 0:CT, 0:C],
                    in0=he[:, 0:CT, 0:C],
                    in1=pt[:, 0:CT, :].to_broadcast([128, CT, C]),
                    op=OP.mult)
                nc.vector.tensor_copy(out=rhs[:, 0:CT, C:FR2], in_=pt[:, 0:CT, :])

                for lb in range(CB):
                    db = db0 + lb
                    acc = ps.tile([128, FR2], F32, tag="acc2")
                    for j in range(B):
                        jj = lb * B + j
                        nc.tensor.matmul(acc[:], lhsT=S[:, jj, :],
                                         rhs=rhs[:, jj, :],
                                         start=(j == 0), stop=(j == B - 1))
                    rec = drsb.tile([128, 1], F32, tag="rec2")
                    nc.vector.tensor_scalar_add(rec[:], acc[:, C:FR2], 1e-16)
                    nc.vector.reciprocal(rec[:], rec[:])
                    o2 = drsb.tile([128, C], F32, tag="o2")
                    nc.vector.tensor_tensor(
                        out=o2[:], in0=acc[:, 0:C],
                        in1=rec[:].to_broadcast([128, C]), op=OP.mult)
                    nc.vector.tensor_add(out=o2[:], in0=o2[:], in1=b2rep[:])
                    mneg = drsb.tile([128, 1], F32, tag="mneg")
                    nc.vector.tensor_reduce(out=mneg[:], in_=o2[:], axis=AX.X,
                                            op=OP.max, negate=True)
                    escr = drsb.tile([128, C], F32, tag="escr")
                    ssum = drsb.tile([128, 1], F32, tag="ssum")
                    nc.scalar.activation(escr[:], o2[:], AF.Exp,
                                         bias=mneg[:, 0:1], accum_out=ssum[:])
                    lns = drsb.tile([128, 1], F32, tag="lns")
                    nc.scalar.activation(lns[:], ssum[:], AF.Ln)
                    tsh = drsb.tile([128, 1], F32, tag="tsh")
                    nc.vector.tensor_sub(out=tsh[:], in0=mneg[:], in1=lns[:])
                    fin = drsb.tile([128, C], F32, tag="fin")
                    nc.vector.tensor_tensor(out=fin[:], in0=o2[:],
                                            in1=tsh[:].to_broadcast([128, C]),
                                            op=OP.add)
                    nv = 128 if db < NB - 1 else LASTV
                    nc.sync.dma_start(out=out_d[db * 128:db * 128 + nv, :],
                                      in_=fin[:nv, :])

    nc.compile()
    return nc


_NC_CACHE: dict = {}


def _get_nc(cfg: Cfg):
    if cfg not in _NC_CACHE:
        _NC_CACHE[cfg] = build_nc(cfg)
    return _NC_CACHE[cfg]


def kernel(x, edge_index, W1, a_src1, a_dst1, b1, W2, a_src2, a_dst2, b2,
           cfg: Cfg | None = None, _run=None):
    x = np.asarray(x)
    edge_index = np.asarray(edge_index)
    if cfg is None:
        cfg = Cfg()
        loops = np.arange(cfg.N, dtype=np.int64)
        dst = np.concatenate([np.asarray(edge_index[1]).astype(np.int64), loops])
        b = compute_B(dst, cfg)
        if b != cfg.B:
            cfg = Cfg(B=b)
    in_maps = build_in_maps(x, edge_index, W1, a_src1, a_dst1, b1,
                            W2, a_src2, a_dst2, b2, cfg)
    nc = _get_nc(cfg)
    if _run is not None:
        results = _run(nc, in_maps)
    else:
        res = run_bass_kernel_spmd(nc, in_maps, list(range(cfg.NC)))
        results = res.results
    out = np.concatenate([results[c]["out"] for c in range(cfg.NC)], axis=0)
    return out.astype(np.float32)
